# revision 53
# baseline (speedup 1.0000x reference)
"""Trainium2 Bass kernel for a causal-EMA encoder:

    out = EMA3(x @ W_down^T) @ W_up^T

with EMA layer i:  y_t = a_i * y_{t-1} + (1 - a_i) * h_t,  a_i = sigmoid(log_a[i]).

Shapes (hardcoded): x [4, 4096, 2048], W_down [512, 2048], W_up [2048, 512],
log_a [3, 512]. Output [4, 4096, 2048] fp32.

Strategy (8 NeuronCores, SPMD, no collectives):
  * Shard (batch, sequence-half): core c handles batch c//2, L-half c%2.
    Second-half cores recompute a KWARM-token warmup prefix instead of
    communicating scan state (decay a ~ 0.95 makes history die geometrically).
  * All matmuls run in fp8e4 with MatmulPerfMode.DoubleRow (2 contraction
    slots per instruction at 0.5 PE cycles per output row). Each operand is
    split into hi + lo fp8 planes (hi = fp8(v), lo = fp8(v - hi)), and each
    product keeps the three big terms hi*hi + hi*lo + lo*hi:
      - down-proj: per k-pair one hi*hi instr; per k-tile one mixed instr
        computing w_lo^T x_hi + w_hi^T x_lo in its two slots.
      - up-proj: per k-tile one instr with duplicated-hi weights computing
        wu_hi^T (y_hi + y_lo); per k-pair one wu_lo^T y_hi instr.
    This is 0.75x the PE time of an fp16 kernel with ~0.25% end-to-end error.
  * fp8 needs power-of-2 pre-scaling so the lo planes stay out of subnormals:
    W* x64, x x8, y x8. The inverses fold into the existing per-channel PSUM
    evacuation scale (prod(1-a)/64) and a free host-side 2^-9 on the output.
  * The three EMA input injections fold into one pre-scale prod_i(1-a_i)
    (linearity), so the scans are pure a-decay TensorTensorScan ops on DVE.
  * Output is stored fp16 (values are 512*out, well within range; host
    converts and rescales), halving the store traffic.
  * Schedule: x arrives per-chunk (exact-width DRAM tensors so every DMA is
    >=512B-contiguous); early chunks run the down-proj k-outer so the PE
    consumes DMA pieces as they land; the up-proj trails the down-proj by one
    chunk; the final chunk evacuates half-width PSUM groups so the trailing
    evacuate+store chain is short.
"""

import sys

for _p in ("/opt/trn_rl_repo", "/root/.axon_site/_ro/trn_rl_repo"):
    if _p not in sys.path:
        sys.path.append(_p)

import numpy as np
import ml_dtypes
from contextlib import ExitStack

import concourse.tile as tile
from concourse import bacc, mybir
from concourse.bass_utils import run_bass_kernel_spmd

B, L, D, DI, NL = 4, 4096, 2048, 512, 3
P = 128
N_CORES = 8
HALF = L // 2          # tokens produced per core
KWARM = 256            # recomputed warmup tokens on second-half cores
LC = HALF + KWARM
CHUNK = 512            # max l-chunk (= fp32 PSUM bank free dim)
NKD = D // P           # 16 k-tiles for down-proj
NME = DI // P          # 4  e-tiles (down-proj m / up-proj k)
NMD = D // P           # 16 d-tiles for up-proj

FP16 = mybir.dt.float16
FP8 = mybir.dt.float8e4
F32 = mybir.dt.float32
MULT = mybir.AluOpType.mult
ADD = mybir.AluOpType.add
SUB = mybir.AluOpType.subtract
DR = mybir.MatmulPerfMode.DoubleRow

F8NP = ml_dtypes.float8_e4m3

WIDTHS = [KWARM, 512, 512, 512, 512]   # warm + 2048 output tokens
WARM_CHUNKS = 1
KOUTER_CHUNKS = 2      # chunks emitted k-outer (consume x pieces as they land)
NCH = len(WIDTHS)
L0S = [0]
for _w in WIDTHS:
    L0S.append(L0S[-1] + _w)

_module_cache: dict[str, object] = {}
LAST_RESULTS = None  # BassKernelResults of the most recent run (for profiling)


def _build_body(ctx: ExitStack, tc: tile.TileContext):
    nc = tc.nc

    xds = [
        nc.dram_tensor(f"x8_{j}", [D, 2, WIDTHS[j]], FP8, kind="ExternalInput")
        .ap()
        .rearrange("(kt p) hl l -> p kt hl l", p=P)
        for j in range(NCH)
    ]
    wd8 = nc.dram_tensor("wd8", [D, 2, DI], FP8, kind="ExternalInput").ap()
    wu8 = nc.dram_tensor("wu8", [DI, 2, D], FP8, kind="ExternalInput").ap()
    dec = nc.dram_tensor("dec", [NME, P, NL], F32, kind="ExternalInput").ap()
    sc = nc.dram_tensor("sc", [NME, P, 1], F32, kind="ExternalInput").ap()
    outT = nc.dram_tensor("outT", [D, HALF], FP16, kind="ExternalOutput").ap()

    singles = ctx.enter_context(tc.tile_pool(name="singles", bufs=1))
    xpool = ctx.enter_context(tc.tile_pool(name="xpool", bufs=3))
    opool = ctx.enter_context(tc.tile_pool(name="opool", bufs=8))
    psum = ctx.enter_context(tc.tile_pool(name="psum", bufs=8, space="PSUM"))

    # ---- persistent weights / constants ----
    wd_sb = singles.tile([P, NKD, 2, DI], FP8)     # hl = (lo, hi)
    wu_sb = singles.tile([P, NME, 2, D], FP8)      # hl = (hi, lo)
    dec_sb = singles.tile([P, NME, NL], F32)
    sc_sb = singles.tile([P, NME, 1], F32)

    # per-(e-tile, layer) decay rows broadcast along the chunk
    ones = singles.tile([P, CHUNK], F32)
    a_sb = singles.tile([P, NME, NL, CHUNK], F32)

    # scan chain tiles: explicit double-generation so chunk j+1's scan can
    # take its carry directly from chunk j's output tile (no carry copies)
    hsc_t = [
        [singles.tile([P, CHUNK], F32, name=f"hsc_{m}_{g}") for g in range(2)]
        for m in range(NME)
    ]
    z_t = [
        [
            [singles.tile([P, CHUNK], F32, name=f"z{i}_{m}_{g}") for g in range(2)]
            for m in range(NME)
        ]
        for i in range(NL)
    ]
    y8_t = [singles.tile([P, NME, 2, CHUNK], FP8, name=f"y8_{g}") for g in range(2)]

    wd8r = wd8.rearrange("(kt p) hl e -> p kt hl e", p=P)
    wu8r = wu8.rearrange("(kt p) hl d -> p kt hl d", p=P)
    outTr = outT.rearrange("(mt p) l -> p mt l", p=P)

    def emit_consts():
        # tiny DMAs + decay broadcasts on DVE while the big DMAs stream
        nc.sync.dma_start(out=dec_sb, in_=dec.rearrange("t p l -> p t l"))
        nc.sync.dma_start(out=sc_sb, in_=sc.rearrange("t p o -> p t o"))
        nc.vector.memset(ones, 1.0)
        for t in range(NME):
            for i in range(NL):
                nc.vector.tensor_scalar_mul(
                    a_sb[:, t, i, :], ones, dec_sb[:, t, i : i + 1]
                )

    x_sbs = {}

    def emit_xdma(j: int, interleave=(), pieces=((0, 2), (2, 2), (4, 4), (8, 4), (12, 4))):
        """DMA chunk j's x in k-tile pieces, optionally interleaving other
        (weight) DMA thunks between pieces."""
        w = WIDTHS[j]
        x_sb = xpool.tile([P, NKD, 2, w], FP8, tag="x", name=f"x_sb_{j}")
        x_sbs[j] = x_sb
        others = list(interleave)
        for pi, (p0, szk) in enumerate(pieces):
            if pi < len(others):
                others[pi]()
            nc.sync.dma_start(
                out=x_sb[:, p0 : p0 + szk], in_=xds[j][:, p0 : p0 + szk]
            )
        for o in others[len(pieces):]:
            o()

    def down_matmul(ph, x_sb, m, kp_or_k, cross, w, first, last):
        ms = m * P
        if not cross:
            ks = slice(2 * kp_or_k, 2 * kp_or_k + 2)
            nc.tensor.matmul(
                ph[:, :w],
                lhsT=wd_sb[:, ks, 1, ms : ms + P],
                rhs=x_sb[:, ks, 0, :w],
                start=first, stop=last, perf_mode=DR,
            )
        else:
            k = kp_or_k
            nc.tensor.matmul(
                ph[:, :w],
                lhsT=wd_sb[:, k, :, ms : ms + P],
                rhs=x_sb[:, k, :, :w],
                start=first, stop=last, perf_mode=DR,
            )

    def emit_down_tail(j, m, ph):
        """PSUM evacuate + scans + y8 split for one m-tile of chunk j."""
        w = WIDTHS[j]
        g = j % 2
        hsc = hsc_t[m][g]
        nc.scalar.mul(hsc[:, :w], ph[:, :w], sc_sb[:, m, 0:1])
        zin = hsc
        for i in range(NL):
            zt = z_t[i][m][g]
            if j == 0:
                init = 0.0
            else:
                wprev = WIDTHS[j - 1]
                init = z_t[i][m][1 - g][:, wprev - 1 : wprev]
            nc.vector.tensor_tensor_scan(
                zt[:, :w], a_sb[:, m, i, :w], zin[:, :w],
                initial=init, op0=MULT, op1=ADD,
            )
            zin = zt
        if j >= WARM_CHUNKS:
            y8 = y8_t[g]
            nc.scalar.copy(out=y8[:, m, 0, :w], in_=zin[:, :w])
            nc.vector.tensor_tensor(
                out=y8[:, m, 1, :w], in0=zin[:, :w], in1=y8[:, m, 0, :w], op=SUB,
            )

    def emit_down_range(j, phs, klo, khi):
        # k-outer, cross-term first per k-tile: each matmul needs only the
        # x/wd pieces holding its k-tile, so PE consumes DMA as it lands
        w = WIDTHS[j]
        x_sb = x_sbs[j]
        for k in range(klo, khi):
            for m in range(NME):
                down_matmul(phs[m], x_sb, m, k, True, w, k == 0, False)
            if k % 2 == 1:
                kp = k // 2
                for m in range(NME):
                    down_matmul(
                        phs[m], x_sb, m, kp, False, w, False, k == NKD - 1
                    )

    def alloc_phs(j):
        return [
            psum.tile([P, CHUNK], F32, tag="ps", name=f"ph_{j}_{m}")
            for m in range(NME)
        ]

    def emit_down(j: int):
        w = WIDTHS[j]
        x_sb = x_sbs[j]
        phs = alloc_phs(j)
        if j < KOUTER_CHUNKS:
            emit_down_range(j, phs, 0, NKD)
            for m in range(NME):
                emit_down_tail(j, m, phs[m])
        else:
            for m in range(NME):
                for kp in range(NKD // 2):
                    down_matmul(phs[m], x_sb, m, kp, False, w, kp == 0, False)
                for k in range(NKD):
                    down_matmul(
                        phs[m], x_sb, m, k, True, w, False, k == NKD - 1
                    )
                emit_down_tail(j, m, phs[m])

    def up_matmuls(j, po, mms, w, l0=0, l1=None):
        # per k-tile: wu_hi^T y_hi + wu_lo^T y_hi (stride-0 y_hi in both
        # slots); per k-pair: wu_hi^T y_lo. Ordered kt-pair-progressive
        # so early weight pieces unblock the first instructions.
        y8 = y8_t[j % 2]
        l1 = w if l1 is None else l1
        n = l1 - l0
        for kp in range(NME // 2):
            for k in (2 * kp, 2 * kp + 1):
                yh = y8[:, k, 0, l0:l1]
                nc.tensor.matmul(
                    po[:, l0:l1],
                    lhsT=wu_sb[:, k, :, mms : mms + P],
                    rhs=yh[:, None, :].broadcast_to((P, 2, n)),
                    start=(k == 0), stop=False, perf_mode=DR,
                )
            ks = slice(2 * kp, 2 * kp + 2)
            nc.tensor.matmul(
                po[:, l0:l1],
                lhsT=wu_sb[:, ks, 0, mms : mms + P],
                rhs=y8[:, ks, 1, l0:l1],
                start=False, stop=(kp == NME // 2 - 1), perf_mode=DR,
            )

    def emit_up(j: int):
        w = WIDTHS[j]
        lo = L0S[j] - KWARM
        if j == NCH - 1:
            # Final chunk: half-width PSUM groups (each in its own bank) and
            # per-m-tile single stores. Smaller units drain the trailing
            # evac+store chain ~2x faster, and HWDGE has no competing x
            # prefetch in the last iteration.
            osb2 = None
            for mm in range(NMD):
                mms = mm * P
                single = mm >= NMD - 2
                if single or mm % 2 == 0:
                    osb2 = opool.tile(
                        [P, 2, CHUNK], FP16, tag="osb", name=f"osbf_{j}_{mm}"
                    )
                sl = 0 if single else mm % 2
                last = mm == NMD - 1
                hb = (0, w // 2, w)
                for half in range(2):
                    h0, h1 = hb[half], hb[half + 1]
                    po = psum.tile(
                        [P, CHUNK], F32, tag="ps", name=f"pof_{j}_{mm}_{half}"
                    )
                    up_matmuls(j, po, mms, w, h0, h1)
                    if half == 0:
                        nc.scalar.copy(out=osb2[:, sl, h0:h1], in_=po[:, h0:h1])
                    else:
                        nc.vector.tensor_copy(
                            out=osb2[:, sl, h0:h1], in_=po[:, h0:h1]
                        )
                    if last:
                        # the very last transfer is a short slice, shrinking
                        # the trailing store chain
                        nc.sync.dma_start(
                            out=outTr[:, mm, lo + h0 : lo + h1],
                            in_=osb2[:, 0, h0:h1],
                        )
                if single and not last:
                    nc.sync.dma_start(
                        out=outTr[:, mm, lo : lo + w], in_=osb2[:, 0, :w]
                    )
                elif mm % 2 == 1 and not single:
                    nc.sync.dma_start(
                        out=outTr[:, mm - 1 : mm + 1, lo : lo + w],
                        in_=osb2[:, :, :w],
                    )
            return
        osb2 = None
        for mm in range(NMD):
            mms = mm * P
            po = psum.tile([P, CHUNK], F32, tag="ps", name=f"po_{j}_{mm}")
            up_matmuls(j, po, mms, w)
            # GpSimd cannot read PSUM: alternate evacuations ScalarE/DVE,
            # and store m-tile PAIRS (HWDGE fixed cost is per DMA)
            if mm % 2 == 0:
                osb2 = opool.tile(
                    [P, 2, CHUNK], FP16, tag="osb", name=f"osb_{j}_{mm}"
                )
            if mm % 2 == 1:
                nc.vector.tensor_copy(out=osb2[:, 1, :w], in_=po[:, :w])
                nc.sync.dma_start(
                    out=outTr[:, mm - 1 : mm + 1, lo : lo + w],
                    in_=osb2[:, :, :w],
                )
            else:
                nc.scalar.copy(out=osb2[:, 0, :w], in_=po[:, :w])

    # ---- schedule ----
    # Head: the warm chunk and chunk 1 are emitted with their matmuls
    # interleaved by DMA-piece availability (PE is in-order, so whichever
    # chunk's pieces have landed keeps it busy). DMA stream order:
    #   x0[0:8] wd[0:4] x1[0:8] wd[4:8] x0[8:16] wd[8:16] x1[8:16] consts
    # Then x2+wu, then x(j+1) prefetched one iteration ahead.
    def wd_dma(klo, khi):
        nc.sync.dma_start(out=wd_sb[:, klo:khi], in_=wd8r[:, klo:khi])

    def head_block():
        x0 = xpool.tile([P, NKD, 2, WIDTHS[0]], FP8, tag="x", name="x_sb_0")
        x1 = xpool.tile([P, NKD, 2, WIDTHS[1]], FP8, tag="x", name="x_sb_1")
        x_sbs[0], x_sbs[1] = x0, x1
        wd_dma(0, 2)
        nc.sync.dma_start(out=x0[:, 0:4], in_=xds[0][:, 0:4])
        wd_dma(2, 4)
        nc.sync.dma_start(out=x1[:, 0:4], in_=xds[1][:, 0:4])
        wd_dma(4, 8)
        nc.sync.dma_start(out=x1[:, 4:8], in_=xds[1][:, 4:8])
        nc.sync.dma_start(out=x0[:, 4:8], in_=xds[0][:, 4:8])
        wd_dma(8, 16)
        nc.sync.dma_start(out=x0[:, 8:16], in_=xds[0][:, 8:16])
        emit_consts()
        nc.sync.dma_start(out=x1[:, 8:16], in_=xds[1][:, 8:16])
        ph0 = alloc_phs(0)
        ph1 = alloc_phs(1)
        emit_down_range(0, ph0, 0, 2)
        emit_down_range(0, ph0, 2, 4)
        emit_down_range(1, ph1, 0, 4)
        emit_down_range(1, ph1, 4, 8)
        emit_down_range(0, ph0, 4, 8)
        emit_down_range(0, ph0, 8, NKD)
        for m in range(NME):
            emit_down_tail(0, m, ph0[m])
        emit_down_range(1, ph1, 8, NKD)
        for m in range(NME):
            emit_down_tail(1, m, ph1[m])

    head_block()
    for j in range(2, NCH):
        if j == 2:
            emit_xdma(2, pieces=((0, 8), (8, 8)))
            nc.sync.dma_start(out=wu_sb, in_=wu8r)
        if j + 1 < NCH:
            emit_xdma(j + 1, pieces=((0, 16),))
        emit_down(j)
        if j - 1 >= WARM_CHUNKS:
            emit_up(j - 1)
    emit_up(NCH - 1)


def _pick_kwarm(a=None):
    # kept for compatibility with older harness/test drivers
    return KWARM


def _get_module(kwarm=None):
    if "m" in _module_cache:
        return _module_cache["m"]
    nc = bacc.Bacc("TRN2", target_bir_lowering=False, debug=False, enable_asserts=False)
    with tile.TileContext(nc) as tc:
        with ExitStack() as ctx:
            _build_body(ctx, tc)
    nc.compile()
    _module_cache["m"] = nc
    return nc


def _split8(v: np.ndarray, scale: float):
    """hi/lo fp8 planes of v*scale (pow2 scale keeps lo out of subnormals)."""
    vs = (v * scale).astype(np.float32)
    hi = vs.astype(F8NP)
    lo = (vs - hi.astype(np.float32)).astype(F8NP)
    return hi, lo


def kernel(x, W_down, W_up, log_a):
    global LAST_RESULTS
    x = np.ascontiguousarray(np.asarray(x, dtype=np.float32))
    W_down = np.asarray(W_down, dtype=np.float32)
    W_up = np.asarray(W_up, dtype=np.float32)
    log_a = np.asarray(log_a, dtype=np.float32)
    assert x.shape == (B, L, D) and W_down.shape == (DI, D) and W_up.shape == (D, DI)

    a64 = 1.0 / (1.0 + np.exp(-log_a.astype(np.float64)))          # [NL, DI]
    a = a64.astype(np.float32)
    scale = np.prod(1.0 - a64, axis=0)                             # [DI]

    nc = _get_module()

    wdh, wdl = _split8(np.ascontiguousarray(W_down.T), 64.0)       # [D, DI]
    wd8 = np.ascontiguousarray(np.stack([wdl, wdh], axis=1))       # (lo, hi)
    wuh, wul = _split8(np.ascontiguousarray(W_up.T), 64.0)         # [DI, D]
    wu8_a = np.ascontiguousarray(np.stack([wuh, wul], axis=1))     # (hi, lo)
    dec_a = np.ascontiguousarray(a.T.reshape(NME, P, NL))
    sc_a = np.ascontiguousarray(
        (scale / 64.0).astype(np.float32).reshape(NME, P, 1)
    )

    in_maps = []
    for c in range(N_CORES):
        b, h = divmod(c, 2)
        xt = np.zeros((LC, D), dtype=np.float32)
        lstart = h * HALF - KWARM
        src_lo = max(0, lstart)
        xt[src_lo - lstart :, :] = x[b, src_lo : h * HALF + HALF, :]
        xh, xl = _split8(xt.T, 8.0)                                # [D, LC]
        m = {"wd8": wd8, "wu8": wu8_a, "dec": dec_a, "sc": sc_a}
        for j in range(NCH):
            w = WIDTHS[j]
            xj = np.empty((D, 2, w), dtype=F8NP)
            xj[:, 0, :] = xh[:, L0S[j] : L0S[j] + w]
            xj[:, 1, :] = xl[:, L0S[j] : L0S[j] + w]
            m[f"x8_{j}"] = xj
        in_maps.append(m)

    res = run_bass_kernel_spmd(nc, in_maps, core_ids=list(range(N_CORES)))
    LAST_RESULTS = res

    out = np.empty((B, L, D), dtype=np.float32)
    for c in range(N_CORES):
        b, h = divmod(c, 2)
        o = res.results[c]["outT"].astype(np.float32) * (2.0 ** -9)
        out[b, h * HALF : (h + 1) * HALF, :] = o.T
    return out


# revision 54
# speedup vs baseline: 1.0057x; 1.0057x over previous
"""Trainium2 Bass kernel for a causal-EMA encoder:

    out = EMA3(x @ W_down^T) @ W_up^T

with EMA layer i:  y_t = a_i * y_{t-1} + (1 - a_i) * h_t,  a_i = sigmoid(log_a[i]).

Shapes (hardcoded): x [4, 4096, 2048], W_down [512, 2048], W_up [2048, 512],
log_a [3, 512]. Output [4, 4096, 2048] fp32.

Strategy (8 NeuronCores, SPMD, no collectives):
  * Shard (batch, sequence-half): core c handles batch c//2, L-half c%2.
    Second-half cores recompute a KWARM-token warmup prefix instead of
    communicating scan state (decay a ~ 0.95 makes history die geometrically).
  * All matmuls run in fp8e4 with MatmulPerfMode.DoubleRow (2 contraction
    slots per instruction at 0.5 PE cycles per output row). Each operand is
    split into hi + lo fp8 planes (hi = fp8(v), lo = fp8(v - hi)), and each
    product keeps the three big terms hi*hi + hi*lo + lo*hi:
      - down-proj: per k-pair one hi*hi instr; per k-tile one mixed instr
        computing w_lo^T x_hi + w_hi^T x_lo in its two slots.
      - up-proj: per k-tile one instr with duplicated-hi weights computing
        wu_hi^T (y_hi + y_lo); per k-pair one wu_lo^T y_hi instr.
    This is 0.75x the PE time of an fp16 kernel with ~0.25% end-to-end error.
  * fp8 needs power-of-2 pre-scaling so the lo planes stay out of subnormals:
    W* x64, x x8, y x8. The inverses fold into the existing per-channel PSUM
    evacuation scale (prod(1-a)/64) and a free host-side 2^-9 on the output.
  * The three EMA input injections fold into one pre-scale prod_i(1-a_i)
    (linearity), so the scans are pure a-decay TensorTensorScan ops on DVE.
  * Output is stored fp16 (values are 512*out, well within range; host
    converts and rescales), halving the store traffic.
  * Schedule: x arrives per-chunk (exact-width DRAM tensors so every DMA is
    >=512B-contiguous); early chunks run the down-proj k-outer so the PE
    consumes DMA pieces as they land; the up-proj trails the down-proj by one
    chunk; the final chunk evacuates half-width PSUM groups so the trailing
    evacuate+store chain is short.
"""

import sys

for _p in ("/opt/trn_rl_repo", "/root/.axon_site/_ro/trn_rl_repo"):
    if _p not in sys.path:
        sys.path.append(_p)

import numpy as np
import ml_dtypes
from contextlib import ExitStack

import concourse.tile as tile
from concourse import bacc, mybir
from concourse.bass_utils import run_bass_kernel_spmd

B, L, D, DI, NL = 4, 4096, 2048, 512, 3
P = 128
N_CORES = 8
HALF = L // 2          # tokens produced per core
KWARM = 256            # recomputed warmup tokens on second-half cores
LC = HALF + KWARM
CHUNK = 512            # max l-chunk (= fp32 PSUM bank free dim)
NKD = D // P           # 16 k-tiles for down-proj
NME = DI // P          # 4  e-tiles (down-proj m / up-proj k)
NMD = D // P           # 16 d-tiles for up-proj

FP16 = mybir.dt.float16
FP8 = mybir.dt.float8e4
F32 = mybir.dt.float32
MULT = mybir.AluOpType.mult
ADD = mybir.AluOpType.add
SUB = mybir.AluOpType.subtract
DR = mybir.MatmulPerfMode.DoubleRow

F8NP = ml_dtypes.float8_e4m3

WIDTHS = [KWARM, 512, 512, 512, 512]   # warm + 2048 output tokens
WARM_CHUNKS = 1
KOUTER_CHUNKS = 2      # chunks emitted k-outer (consume x pieces as they land)
NCH = len(WIDTHS)
L0S = [0]
for _w in WIDTHS:
    L0S.append(L0S[-1] + _w)

_module_cache: dict[str, object] = {}
LAST_RESULTS = None  # BassKernelResults of the most recent run (for profiling)


def _build_body(ctx: ExitStack, tc: tile.TileContext):
    nc = tc.nc

    xds = [
        nc.dram_tensor(f"x8_{j}", [D, 2, WIDTHS[j]], FP8, kind="ExternalInput")
        .ap()
        .rearrange("(kt p) hl l -> p kt hl l", p=P)
        for j in range(NCH)
    ]
    wd8 = nc.dram_tensor("wd8", [D, 2, DI], FP8, kind="ExternalInput").ap()
    wu8 = nc.dram_tensor("wu8", [DI, 2, D], FP8, kind="ExternalInput").ap()
    dec = nc.dram_tensor("dec", [NME, P, NL], F32, kind="ExternalInput").ap()
    sc = nc.dram_tensor("sc", [NME, P, 1], F32, kind="ExternalInput").ap()
    outT = nc.dram_tensor("outT", [D, HALF], FP16, kind="ExternalOutput").ap()

    singles = ctx.enter_context(tc.tile_pool(name="singles", bufs=1))
    xpool = ctx.enter_context(tc.tile_pool(name="xpool", bufs=3))
    opool = ctx.enter_context(tc.tile_pool(name="opool", bufs=8))
    psum = ctx.enter_context(tc.tile_pool(name="psum", bufs=8, space="PSUM"))

    # ---- persistent weights / constants ----
    wd_sb = singles.tile([P, NKD, 2, DI], FP8)     # hl = (lo, hi)
    wu_sb = singles.tile([P, NME, 2, D], FP8)      # hl = (hi, lo)
    dec_sb = singles.tile([P, NME, NL], F32)
    sc_sb = singles.tile([P, NME, 1], F32)

    # per-(e-tile, layer) decay rows broadcast along the chunk
    ones = singles.tile([P, CHUNK], F32)
    a_sb = singles.tile([P, NME, NL, CHUNK], F32)

    # scan chain tiles: explicit double-generation so chunk j+1's scan can
    # take its carry directly from chunk j's output tile (no carry copies)
    hsc_t = [
        [singles.tile([P, CHUNK], F32, name=f"hsc_{m}_{g}") for g in range(2)]
        for m in range(NME)
    ]
    z_t = [
        [
            [singles.tile([P, CHUNK], F32, name=f"z{i}_{m}_{g}") for g in range(2)]
            for m in range(NME)
        ]
        for i in range(NL)
    ]
    y8_t = [singles.tile([P, NME, 2, CHUNK], FP8, name=f"y8_{g}") for g in range(2)]

    wd8r = wd8.rearrange("(kt p) hl e -> p kt hl e", p=P)
    wu8r = wu8.rearrange("(kt p) hl d -> p kt hl d", p=P)
    outTr = outT.rearrange("(mt p) l -> p mt l", p=P)

    def emit_consts():
        # tiny DMAs + decay broadcasts on DVE while the big DMAs stream
        nc.sync.dma_start(out=dec_sb, in_=dec.rearrange("t p l -> p t l"))
        nc.sync.dma_start(out=sc_sb, in_=sc.rearrange("t p o -> p t o"))
        nc.vector.memset(ones, 1.0)
        for t in range(NME):
            for i in range(NL):
                nc.vector.tensor_scalar_mul(
                    a_sb[:, t, i, :], ones, dec_sb[:, t, i : i + 1]
                )

    x_sbs = {}

    def emit_xdma(j: int, interleave=(), pieces=((0, 2), (2, 2), (4, 4), (8, 4), (12, 4))):
        """DMA chunk j's x in k-tile pieces, optionally interleaving other
        (weight) DMA thunks between pieces."""
        w = WIDTHS[j]
        x_sb = xpool.tile([P, NKD, 2, w], FP8, tag="x", name=f"x_sb_{j}")
        x_sbs[j] = x_sb
        others = list(interleave)
        for pi, (p0, szk) in enumerate(pieces):
            if pi < len(others):
                others[pi]()
            nc.sync.dma_start(
                out=x_sb[:, p0 : p0 + szk], in_=xds[j][:, p0 : p0 + szk]
            )
        for o in others[len(pieces):]:
            o()

    def down_matmul(ph, x_sb, m, kp_or_k, cross, w, first, last):
        ms = m * P
        if not cross:
            ks = slice(2 * kp_or_k, 2 * kp_or_k + 2)
            nc.tensor.matmul(
                ph[:, :w],
                lhsT=wd_sb[:, ks, 1, ms : ms + P],
                rhs=x_sb[:, ks, 0, :w],
                start=first, stop=last, perf_mode=DR,
            )
        else:
            k = kp_or_k
            nc.tensor.matmul(
                ph[:, :w],
                lhsT=wd_sb[:, k, :, ms : ms + P],
                rhs=x_sb[:, k, :, :w],
                start=first, stop=last, perf_mode=DR,
            )

    def emit_down_tail(j, m, ph):
        """PSUM evacuate + scans + y8 split for one m-tile of chunk j."""
        w = WIDTHS[j]
        g = j % 2
        hsc = hsc_t[m][g]
        nc.scalar.mul(hsc[:, :w], ph[:, :w], sc_sb[:, m, 0:1])
        zin = hsc
        for i in range(NL):
            zt = z_t[i][m][g]
            if j == 0:
                init = 0.0
            else:
                wprev = WIDTHS[j - 1]
                init = z_t[i][m][1 - g][:, wprev - 1 : wprev]
            nc.vector.tensor_tensor_scan(
                zt[:, :w], a_sb[:, m, i, :w], zin[:, :w],
                initial=init, op0=MULT, op1=ADD,
            )
            zin = zt
        if j >= WARM_CHUNKS:
            y8 = y8_t[g]
            nc.scalar.copy(out=y8[:, m, 0, :w], in_=zin[:, :w])
            nc.vector.tensor_tensor(
                out=y8[:, m, 1, :w], in0=zin[:, :w], in1=y8[:, m, 0, :w], op=SUB,
            )

    def emit_down_range(j, phs, klo, khi):
        # k-outer, cross-term first per k-tile: each matmul needs only the
        # x/wd pieces holding its k-tile, so PE consumes DMA as it lands
        w = WIDTHS[j]
        x_sb = x_sbs[j]
        for k in range(klo, khi):
            for m in range(NME):
                down_matmul(phs[m], x_sb, m, k, True, w, k == 0, False)
            if k % 2 == 1:
                kp = k // 2
                for m in range(NME):
                    down_matmul(
                        phs[m], x_sb, m, kp, False, w, False, k == NKD - 1
                    )

    def alloc_phs(j):
        return [
            psum.tile([P, CHUNK], F32, tag="ps", name=f"ph_{j}_{m}")
            for m in range(NME)
        ]

    def emit_down(j: int):
        w = WIDTHS[j]
        x_sb = x_sbs[j]
        phs = alloc_phs(j)
        if j < KOUTER_CHUNKS:
            emit_down_range(j, phs, 0, NKD)
            for m in range(NME):
                emit_down_tail(j, m, phs[m])
        else:
            for m in range(NME):
                for kp in range(NKD // 2):
                    down_matmul(phs[m], x_sb, m, kp, False, w, kp == 0, False)
                for k in range(NKD):
                    down_matmul(
                        phs[m], x_sb, m, k, True, w, False, k == NKD - 1
                    )
                emit_down_tail(j, m, phs[m])

    def up_matmuls(j, po, mms, w, l0=0, l1=None):
        # per k-tile: wu_hi^T y_hi + wu_lo^T y_hi (stride-0 y_hi in both
        # slots); per k-pair: wu_hi^T y_lo. Ordered kt-pair-progressive
        # so early weight pieces unblock the first instructions.
        y8 = y8_t[j % 2]
        l1 = w if l1 is None else l1
        n = l1 - l0
        for kp in range(NME // 2):
            for k in (2 * kp, 2 * kp + 1):
                yh = y8[:, k, 0, l0:l1]
                nc.tensor.matmul(
                    po[:, l0:l1],
                    lhsT=wu_sb[:, k, :, mms : mms + P],
                    rhs=yh[:, None, :].broadcast_to((P, 2, n)),
                    start=(k == 0), stop=False, perf_mode=DR,
                )
            ks = slice(2 * kp, 2 * kp + 2)
            nc.tensor.matmul(
                po[:, l0:l1],
                lhsT=wu_sb[:, ks, 0, mms : mms + P],
                rhs=y8[:, ks, 1, l0:l1],
                start=False, stop=(kp == NME // 2 - 1), perf_mode=DR,
            )

    def emit_up(j: int):
        w = WIDTHS[j]
        lo = L0S[j] - KWARM
        if j == NCH - 1:
            # Final chunk: half-width PSUM groups (each in its own bank) and
            # per-m-tile single stores. Smaller units drain the trailing
            # evac+store chain ~2x faster, and HWDGE has no competing x
            # prefetch in the last iteration.
            osb2 = None
            for mm in range(NMD):
                mms = mm * P
                single = mm >= NMD - 2
                if single or mm % 2 == 0:
                    osb2 = opool.tile(
                        [P, 2, CHUNK], FP16, tag="osb", name=f"osbf_{j}_{mm}"
                    )
                sl = 0 if single else mm % 2
                for half, (h0, h1) in enumerate(((0, w // 2), (w // 2, w))):
                    po = psum.tile(
                        [P, CHUNK], F32, tag="ps", name=f"pof_{j}_{mm}_{half}"
                    )
                    up_matmuls(j, po, mms, w, h0, h1)
                    if half == 0:
                        nc.scalar.copy(out=osb2[:, sl, h0:h1], in_=po[:, h0:h1])
                    else:
                        nc.vector.tensor_copy(
                            out=osb2[:, sl, h0:h1], in_=po[:, h0:h1]
                        )
                if single:
                    nc.sync.dma_start(
                        out=outTr[:, mm, lo : lo + w], in_=osb2[:, 0, :w]
                    )
                elif mm % 2 == 1:
                    nc.sync.dma_start(
                        out=outTr[:, mm - 1 : mm + 1, lo : lo + w],
                        in_=osb2[:, :, :w],
                    )
            return
        osb2 = None
        for mm in range(NMD):
            mms = mm * P
            po = psum.tile([P, CHUNK], F32, tag="ps", name=f"po_{j}_{mm}")
            up_matmuls(j, po, mms, w)
            # GpSimd cannot read PSUM: alternate evacuations ScalarE/DVE,
            # and store m-tile PAIRS (HWDGE fixed cost is per DMA)
            if mm % 2 == 0:
                osb2 = opool.tile(
                    [P, 2, CHUNK], FP16, tag="osb", name=f"osb_{j}_{mm}"
                )
            if mm % 2 == 1:
                nc.vector.tensor_copy(out=osb2[:, 1, :w], in_=po[:, :w])
                nc.sync.dma_start(
                    out=outTr[:, mm - 1 : mm + 1, lo : lo + w],
                    in_=osb2[:, :, :w],
                )
            else:
                nc.scalar.copy(out=osb2[:, 0, :w], in_=po[:, :w])

    # ---- schedule ----
    # Head: the warm chunk and chunk 1 are emitted with their matmuls
    # interleaved by DMA-piece availability (PE is in-order, so whichever
    # chunk's pieces have landed keeps it busy). DMA stream order:
    #   x0[0:8] wd[0:4] x1[0:8] wd[4:8] x0[8:16] wd[8:16] x1[8:16] consts
    # Then x2+wu, then x(j+1) prefetched one iteration ahead.
    def wd_dma(klo, khi):
        nc.sync.dma_start(out=wd_sb[:, klo:khi], in_=wd8r[:, klo:khi])

    def head_block():
        x0 = xpool.tile([P, NKD, 2, WIDTHS[0]], FP8, tag="x", name="x_sb_0")
        x1 = xpool.tile([P, NKD, 2, WIDTHS[1]], FP8, tag="x", name="x_sb_1")
        x_sbs[0], x_sbs[1] = x0, x1
        wd_dma(0, 2)
        nc.sync.dma_start(out=x0[:, 0:4], in_=xds[0][:, 0:4])
        wd_dma(2, 4)
        nc.sync.dma_start(out=x1[:, 0:4], in_=xds[1][:, 0:4])
        wd_dma(4, 8)
        nc.sync.dma_start(out=x1[:, 4:8], in_=xds[1][:, 4:8])
        nc.sync.dma_start(out=x0[:, 4:8], in_=xds[0][:, 4:8])
        wd_dma(8, 16)
        nc.sync.dma_start(out=x0[:, 8:16], in_=xds[0][:, 8:16])
        emit_consts()
        nc.sync.dma_start(out=x1[:, 8:16], in_=xds[1][:, 8:16])
        ph0 = alloc_phs(0)
        ph1 = alloc_phs(1)
        emit_down_range(0, ph0, 0, 2)
        emit_down_range(0, ph0, 2, 4)
        emit_down_range(1, ph1, 0, 4)
        emit_down_range(1, ph1, 4, 8)
        emit_down_range(0, ph0, 4, 8)
        emit_down_range(0, ph0, 8, NKD)
        for m in range(NME):
            emit_down_tail(0, m, ph0[m])
        emit_down_range(1, ph1, 8, NKD)
        for m in range(NME):
            emit_down_tail(1, m, ph1[m])

    head_block()
    for j in range(2, NCH):
        if j == 2:
            emit_xdma(2, pieces=((0, 8), (8, 8)))
            nc.sync.dma_start(out=wu_sb, in_=wu8r)
        if j + 1 < NCH:
            emit_xdma(j + 1, pieces=((0, 16),))
        emit_down(j)
        if j - 1 >= WARM_CHUNKS:
            emit_up(j - 1)
    emit_up(NCH - 1)


def _pick_kwarm(a=None):
    # kept for compatibility with older harness/test drivers
    return KWARM


def _get_module(kwarm=None):
    if "m" in _module_cache:
        return _module_cache["m"]
    nc = bacc.Bacc("TRN2", target_bir_lowering=False, debug=False, enable_asserts=False)
    with tile.TileContext(nc) as tc:
        with ExitStack() as ctx:
            _build_body(ctx, tc)
    nc.compile()
    _module_cache["m"] = nc
    return nc


def _split8(v: np.ndarray, scale: float):
    """hi/lo fp8 planes of v*scale (pow2 scale keeps lo out of subnormals)."""
    vs = (v * scale).astype(np.float32)
    hi = vs.astype(F8NP)
    lo = (vs - hi.astype(np.float32)).astype(F8NP)
    return hi, lo


def kernel(x, W_down, W_up, log_a):
    global LAST_RESULTS
    x = np.ascontiguousarray(np.asarray(x, dtype=np.float32))
    W_down = np.asarray(W_down, dtype=np.float32)
    W_up = np.asarray(W_up, dtype=np.float32)
    log_a = np.asarray(log_a, dtype=np.float32)
    assert x.shape == (B, L, D) and W_down.shape == (DI, D) and W_up.shape == (D, DI)

    a64 = 1.0 / (1.0 + np.exp(-log_a.astype(np.float64)))          # [NL, DI]
    a = a64.astype(np.float32)
    scale = np.prod(1.0 - a64, axis=0)                             # [DI]

    nc = _get_module()

    wdh, wdl = _split8(np.ascontiguousarray(W_down.T), 64.0)       # [D, DI]
    wd8 = np.ascontiguousarray(np.stack([wdl, wdh], axis=1))       # (lo, hi)
    wuh, wul = _split8(np.ascontiguousarray(W_up.T), 64.0)         # [DI, D]
    wu8_a = np.ascontiguousarray(np.stack([wuh, wul], axis=1))     # (hi, lo)
    dec_a = np.ascontiguousarray(a.T.reshape(NME, P, NL))
    sc_a = np.ascontiguousarray(
        (scale / 64.0).astype(np.float32).reshape(NME, P, 1)
    )

    in_maps = []
    for c in range(N_CORES):
        b, h = divmod(c, 2)
        xt = np.zeros((LC, D), dtype=np.float32)
        lstart = h * HALF - KWARM
        src_lo = max(0, lstart)
        xt[src_lo - lstart :, :] = x[b, src_lo : h * HALF + HALF, :]
        xh, xl = _split8(xt.T, 8.0)                                # [D, LC]
        m = {"wd8": wd8, "wu8": wu8_a, "dec": dec_a, "sc": sc_a}
        for j in range(NCH):
            w = WIDTHS[j]
            xj = np.empty((D, 2, w), dtype=F8NP)
            xj[:, 0, :] = xh[:, L0S[j] : L0S[j] + w]
            xj[:, 1, :] = xl[:, L0S[j] : L0S[j] + w]
            m[f"x8_{j}"] = xj
        in_maps.append(m)

    res = run_bass_kernel_spmd(nc, in_maps, core_ids=list(range(N_CORES)))
    LAST_RESULTS = res

    out = np.empty((B, L, D), dtype=np.float32)
    for c in range(N_CORES):
        b, h = divmod(c, 2)
        o = res.results[c]["outT"].astype(np.float32) * (2.0 ** -9)
        out[b, h * HALF : (h + 1) * HALF, :] = o.T
    return out
